# revision 21
# baseline (speedup 1.0000x reference)
"""ColorAttention Trainium2 kernel.

Data-parallel over batch: core b handles batch element b.
Per core:
  - mask [256,256,313] is cast to fp8 (0/1 values, lossless) on the host and
    streamed from HBM (20.9MB with c padded to 320), then patch-reduced via
    block-diagonal ones matmuls on the PE in fp8 DoubleRow mode (2 image
    columns per PE cycle, PSUM accumulation), giving m[s,c] = sum over 16x16
    patch. Multiplicative attention mask is_one(m) = relu(1-(m-1)^2)
    (exact for integer m: 1 iff m==1).
  - attention computed in transposed layout throughout:
      qkvT[f,n] = sum_e qkv_wT[e,f] * inputsT[e,n]
      scoresT[m,n] = sum_d kT[d,m] qT[d,n];  expT = exp(scoresT/tau) * mask
      outT_aug[d|1,n] = sum_m v_aug[m,d|1] expT[m,n]   (row 64 = denom)
      out[n,g] = (sum_{h,d} (outT_h/denom_h)[d,n] o_wT[h*64+d,g]) + o_b
  - all attention matmuls in bf16 (1 cyc/col at any width); heads packed in
    pairs on the 128 partitions for normalize / o_proj.
  - setup DMAs ride the idle SP HWDGE ring (bd first) so the ACT engine is
    free for exp and the mask stream (gpsimd SWDGE ring) is unobstructed.
  - per-head softmax denominators are ACT-copied from psum row 64 straight to
    partitions {0,32,64,96} of a gather tile; one DVE reciprocal per 4 heads;
    PE broadcasts each recip row into the matching 64-partition half of a
    [128,570] psum so one DVE mul normalizes a head pair in place.
"""

import numpy as np
import ml_dtypes

# tolerate environments without the optional NTFF profile hook module when
# BASS_TRACE is set externally
try:
    import antenv.axon_hooks  # noqa: F401
except Exception:
    import sys as _sys
    import types as _types
    _m = _types.ModuleType("antenv.axon_hooks")
    _m.set_axon_ntff_profile_hook = lambda h: None
    _m.get_axon_ntff_profile_hook = lambda: None
    try:
        import antenv
        antenv.axon_hooks = _m
        _sys.modules["antenv.axon_hooks"] = _m
    except Exception:
        pass

import concourse.bass as bass
import concourse.mybir as mybir
import concourse.tile as tile
from concourse import bacc
from concourse.bass_utils import run_bass_kernel_spmd

F32 = mybir.dt.float32
F32R = mybir.dt.float32r
BF16 = mybir.dt.bfloat16
FP8 = mybir.dt.float8e4
AFT = mybir.ActivationFunctionType
DR = mybir.MatmulPerfMode.DoubleRow

B = 8
SEQ = 256
NCLS = 313
NCP = 320  # c dim padded to a 16B multiple so fp8 DoubleRow strides are legal
E = 512
NH = 8
HD = 64
N1 = SEQ + NCLS  # 569
P = 16
IMG = 256

# n/m chunking of the 569 token dim.
N1P = 570
CHUNKS = [(0, 128), (128, 128), (256, 128), (384, 128), (512, 57)]
CWP = [128, 128, 128, 128, 58]
SPANS = [(0, 512), (512, 58)]

# mask stream tiling: (row_block, col0, width). Small lead-in tiles so the
# first PE work starts early; 64-col (2.62MB) tiles once the pipe is primed.
# The first four tiles ride the sync HWDGE ring (which starves the SWDGE ring
# while active, so urgent data must go there); the rest stream on SWDGE.
TILES = [
    (0, 0, 16), (0, 16, 16), (0, 32, 32), (0, 64, 64), (0, 128, 64),
    (0, 192, 64),
    (1, 0, 64), (1, 64, 64), (1, 128, 64), (1, 192, 64),
]
N_SYNC_TILES = 7
# attention work units interleaved after each tile's matmuls (33 total)
BUDGET = [2, 1, 2, 3, 5, 5, 5, 5, 5, 0]
# scr-based keep-warm matmuls after early tiles (no data deps)
FILLER = [2, 0, 0, 0, 0, 0, 0, 0, 0, 0]

LAST_RESULT = None
_CACHED = {}


def r32(ap):
    if ap.dtype == F32R:
        return ap
    return ap.bitcast(F32R)


def _build_program():
    nc = bacc.Bacc("TRN2", target_bir_lowering=False, debug=False, num_devices=B)

    # ---- DRAM I/O ----
    d_xT = nc.dram_tensor("xT", [128, 4 * N1P], BF16, kind="ExternalInput").ap()
    d_mask = nc.dram_tensor("mask", [IMG, IMG * NCP], FP8, kind="ExternalInput").ap()
    d_qkvwT = nc.dram_tensor("qkv_wT", [128, 4 * 3 * E], BF16, kind="ExternalInput").ap()
    d_owP = nc.dram_tensor("o_wP", [128, 4 * E], BF16, kind="ExternalInput").ap()
    d_ob = nc.dram_tensor("o_b", [1, E], F32, kind="ExternalInput").ap()
    d_tau = nc.dram_tensor("tau", [1, 1], F32, kind="ExternalInput").ap()
    d_bd = nc.dram_tensor("bd", [128, 16 * 256], FP8, kind="ExternalInput").ap()
    d_ident = nc.dram_tensor("ident", [128, 128], BF16, kind="ExternalInput").ap()
    d_out = nc.dram_tensor("out", [N1, E], BF16, kind="ExternalOutput").ap()

    with tile.TileContext(nc) as tc:
        _emit(nc, tc, d_xT, d_mask, d_qkvwT, d_owP, d_ob, d_tau, d_bd, d_ident, d_out)

    nc.compile()
    return nc


def _emit(nc, tc, d_xT, d_mask, d_qkvwT, d_owP, d_ob, d_tau, d_bd, d_ident, d_out):
    from contextlib import ExitStack

    ctx = ExitStack()
    singles = ctx.enter_context(tc.tile_pool(name="singles", bufs=1))
    expool = ctx.enter_context(tc.tile_pool(name="expT", bufs=40))
    opool = ctx.enter_context(tc.tile_pool(name="outTsb", bufs=4))
    spool = ctx.enter_context(tc.tile_pool(name="smalls", bufs=2))
    ps_a = ctx.enter_context(tc.tile_pool(name="ps_a", bufs=2, space="PSUM"))
    ps_big = ctx.enter_context(tc.tile_pool(name="ps_big", bufs=2, space="PSUM"))
    mctx = ExitStack()
    mh_small = mctx.enter_context(tc.tile_pool(name="mh_small", bufs=2))
    mh_mid = mctx.enter_context(tc.tile_pool(name="mh_mid", bufs=1))
    mpool = mctx.enter_context(tc.tile_pool(name="mask_stream", bufs=3))
    ps_mask = mctx.enter_context(tc.tile_pool(name="ps_mask", bufs=2, space="PSUM"))

    # ---- persistent SBUF ----
    inputsT4 = singles.tile([128, 4, N1P], BF16, tag="inT", name="inputsT4")
    inputsT = [inputsT4[:, i, :] for i in range(4)]
    qkvwT4 = singles.tile([128, 4, 3 * E], BF16, tag="qkvwT", name="qkvwT4")
    qkvwT = [qkvwT4[:, i, :] for i in range(4)]
    owP = singles.tile([128, 4, E], BF16, tag="owP", name="owP")
    bd_sb = singles.tile([128, 16, 2, 128], FP8, tag="bd", name="bd_sb")
    ident_sb = singles.tile([128, 128], BF16, tag="ident", name="ident_sb")
    ones_sb = singles.tile([128, 64], BF16, tag="ones", name="ones_sb")
    rtau = singles.tile([128, 1], F32, tag="rtau", name="rtau")
    ob_bc = singles.tile([128, E], F32, tag="ob", name="ob_bc")
    qkT = [singles.tile([128, N1P], BF16, tag=f"qkT{i}", name=f"qkT{i}") for i in range(8)]
    v_sb = [singles.tile([128, NH, HD + 1], BF16, tag=f"vsb{i}", name=f"v_sb{i}") for i in range(5)]
    isone = [singles.tile([128, NCLS], BF16, tag=f"iso{i}", name=f"isone{i}") for i in range(2)]
    isoT = [singles.tile([128, SEQ], BF16, tag=f"isoT{i}", name=f"isoT{i}") for i in range(3)]
    den4 = [singles.tile([128, N1P], F32, tag=f"den{g}", name=f"den4_{g}") for g in range(2)]
    drec_f = singles.tile([128, N1P], F32, tag="drecf", name="drec_f")
    drec = [singles.tile([128, N1P], BF16, tag=f"drec{g}", name=f"drec{g}") for g in range(2)]

    # ---- short HAM warmup: keep the PE busy while the setup DMAs and the
    # first mask tiles land (the HAM SHORT window needs ~3.4us of activity) ----
    scr = singles.tile([128, 640], BF16, tag="scr", name="scr")
    nc.vector.memset(scr, 1.0)
    ps_warm = ps_a.tile([128, 512], F32, tag="psa", name="ps_warm")
    for _ in range(6):
        nc.tensor.matmul(out=ps_warm, lhsT=scr[:, 0:128], rhs=scr[:, 128:640],
                         start=True, stop=True)

    # ---- the two broadcast loads (tau, o_b need partition-replication ->
    # SWDGE) lead the gpsimd ring ----
    tau_bc = bass.AP(tensor=d_tau.tensor, offset=d_tau.offset, ap=[[0, 128], [1, 1]])
    tau_sb = singles.tile([128, 1], F32, tag="tau", name="tau_sb")
    nc.gpsimd.dma_start(out=tau_sb, in_=tau_bc)
    ob_src = bass.AP(tensor=d_ob.tensor, offset=d_ob.offset, ap=[[0, 128], [1, E]])
    nc.gpsimd.dma_start(out=ob_bc, in_=ob_src)
    nc.vector.reciprocal(out=rtau, in_=tau_sb)
    nc.vector.memset(ones_sb, 1.0)
    neg1 = singles.tile([128, 1], F32, tag="neg1", name="neg1")
    nc.vector.memset(neg1, -1.0)

    def mask_tile_dma(idx):
        rt, c0, w = TILES[idx]
        if w == 16:
            t = mh_small.tile([128, 1, 8, 2, NCP], FP8, tag="mh_s", name="mh_s")
        elif w == 32:
            t = mh_mid.tile([128, 2, 8, 2, NCP], FP8, tag="mh_m", name="mh_m")
        else:
            t = mpool.tile([128, 4, 8, 2, NCP], FP8, tag="mstream", name="mstream")
        src = bass.AP(
            tensor=d_mask.tensor,
            offset=d_mask.offset + rt * 128 * IMG * NCP + c0 * NCP,
            ap=[[IMG * NCP, 128], [1, w * NCP]],
        )
        eng = nc.sync if idx < N_SYNC_TILES else nc.gpsimd
        eng.dma_start(out=t[:, : w // 16], in_=src)
        return t

    # ---- DMA issue order. The two rings share SDMA bandwidth roughly
    # fairly, so everything needed early rides the sync HWDGE ring in exact
    # consumption order; the SWDGE ring carries only the steady-state tail
    # tiles, whose issue is naturally delayed by mask-pool slot recycling.
    pre = {}
    nc.sync.dma_start(out=bd_sb, in_=d_bd)
    pre[0] = mask_tile_dma(0)
    nc.sync.dma_start(out=inputsT4, in_=d_xT)
    nc.sync.dma_start(out=qkvwT4, in_=d_qkvwT)
    pre[1] = mask_tile_dma(1)
    pre[2] = mask_tile_dma(2)
    pre[3] = mask_tile_dma(3)
    nc.sync.dma_start(out=owP, in_=d_owP)
    nc.sync.dma_start(out=ident_sb, in_=d_ident)
    pre[4] = mask_tile_dma(4)
    pre[5] = mask_tile_dma(5)

    # ---- attention work units (emitted interleaved with the mask stream) ----
    expT = {}

    def unit_qkvT(fc):
        def go():
            for s0, sw in SPANS:
                ps = ps_a.tile([128, sw], F32, tag="psa", name="pswork")
                for ec in range(4):
                    nc.tensor.matmul(
                        out=ps,
                        lhsT=qkvwT[ec][:, fc * 128:(fc + 1) * 128],
                        rhs=inputsT[ec][:, s0:s0 + sw],
                        start=(ec == 0), stop=(ec == 3),
                    )
                with nc.allow_low_precision(reason="bf16 qk"):
                    nc.vector.tensor_copy(out=qkT[fc][:, s0:s0 + sw], in_=ps)
        return go

    def unit_v(mc):
        def go():
            c0, cw = CHUNKS[mc]
            cwp = CWP[mc]
            ps = ps_a.tile([128, NH, HD], F32, tag="psa", name="pswork")
            for ec in range(4):
                nc.tensor.matmul(
                    out=ps[:cwp],
                    lhsT=inputsT[ec][:, c0:c0 + cwp],
                    rhs=qkvwT[ec][:, 2 * E:3 * E],
                    start=(ec == 0), stop=(ec == 3),
                )
            with nc.allow_low_precision(reason="bf16 v"):
                nc.vector.tensor_copy(out=v_sb[mc][:cw, :, 0:HD], in_=ps[:cw])
            nc.vector.memset(v_sb[mc][:cw, :, HD:HD + 1], 1.0)
        return go

    def unit_scores_pair(q, mc):
        # heads 2q (PE rows 0:64) and 2q+1 (rows 64:128): the two matmuls of a
        # span are emitted adjacently so their disjoint row groups run
        # concurrently on the array.
        def go():
            c0, cw = CHUNKS[mc]
            cwp = CWP[mc]
            kt = qkT[4 + q]
            qt = qkT[q]
            ets = []
            pss = []
            for hh in range(2):
                et = expool.tile([128, N1P], BF16, tag="expT", name="expT")
                expT[(2 * q + hh, mc)] = et
                ets.append(et)
                pss.append(ps_big.tile([128, N1P], F32, tag="big", name="ps_sc"))
            for s0, sw in SPANS:
                for hh in range(2):
                    hb = 64 * hh
                    nc.tensor.matmul(
                        out=pss[hh][:cwp, s0:s0 + sw],
                        lhsT=kt[hb:hb + 64, c0:c0 + cwp],
                        rhs=qt[hb:hb + 64, s0:s0 + sw],
                        start=True, stop=True,
                    )
            for hh in range(2):
                nc.scalar.activation(
                    out=ets[hh][:cwp, :], in_=pss[hh][:cwp, :],
                    func=AFT.Exp, scale=rtau[:cwp],
                )
        return go

    # interleave so scores (ACT exp) work spreads across the whole stream
    units = []
    for q in range(4):
        units.append(unit_qkvT(q))
        units.append(unit_qkvT(4 + q))
        units.append(unit_v(q))
        for mc in range(5):
            units.append(unit_scores_pair(q, mc))
    units.append(unit_v(4))

    # ---- is_one computation (psum -> multiplicative mask) ----
    ps_m = [None, None]

    def emit_isone(i):
        tmp = spool.tile([128, NCLS], F32, tag="isotmp", name="isotmp")
        nc.scalar.activation(out=tmp, in_=ps_m[i], func=AFT.Square, bias=neg1)
        nc.scalar.activation(out=isone[i], in_=tmp, func=AFT.Relu, scale=-1.0, bias=1.0)

    def emit_isoT(i):
        # transpose is_one -> isoT (c on partitions); half i fills columns
        # i*128..i*128+128
        for j in range(3):
            cw = 57 if j == 2 else 128
            pst = ps_a.tile([128, 128], BF16, tag="psa", name="pswork_t")
            nc.tensor.transpose(out=pst[:cw, :], in_=isone[i][:, j * 128:j * 128 + cw],
                                identity=ident_sb)
            nc.vector.tensor_copy(out=isoT[j][:cw, i * 128:(i + 1) * 128], in_=pst[:cw, :])

    # ---- the mask stream: fp8 tiles of [128 rows, w cols x 320c].
    # DoubleRow pairs adjacent image columns; all matmuls of a row-block
    # accumulate the patch sum into ps_m[rt][s, c].
    ui = 0
    last_rt = -1
    for idx, (rt, c0, w) in enumerate(TILES):
        if rt != last_rt:
            ps_m[rt] = ps_mask.tile([128, NCLS], F32, tag="psmask", name="psmask")
            last_rt = rt
        t = pre.pop(idx, None)
        if t is None:
            t = mask_tile_dma(idx)
        first_tile = (c0 == 0)
        last_tile = (c0 + w == IMG)
        for wi in range(w // 16):
            wg = c0 // 16 + wi
            for jp in range(8):
                nc.tensor.matmul(
                    out=ps_m[rt],
                    lhsT=bd_sb[:, wg, :, :],
                    rhs=t[:, wi, jp, :, :NCLS],
                    start=(first_tile and wi == 0 and jp == 0),
                    stop=(last_tile and wi == w // 16 - 1 and jp == 7),
                    perf_mode=DR,
                )
        for _ in range(FILLER[idx]):
            pf = ps_a.tile([128, 512], F32, tag="psa", name="pf")
            nc.tensor.matmul(out=pf, lhsT=scr[:, 0:128], rhs=scr[:, 128:640],
                             start=True, stop=True)
        if last_tile:
            # is_one first so its ACT ops aren't queued behind the units' exps
            emit_isone(rt)
        for _ in range(BUDGET[idx]):
            if ui < len(units):
                units[ui]()
                ui += 1
        if last_tile:
            emit_isoT(rt)
    while ui < len(units):
        units[ui]()
        ui += 1
    mctx.close()
    ps_nrm = ctx.enter_context(tc.tile_pool(name="ps_nrm", bufs=1, space="PSUM"))

    # ---- mask-mult + attn@v + normalize (head pairs) ----
    # Per head: multiply expT by the mask (color-key side on gpsimd, patch-key
    # side on DVE), attn@v into a 2-bank psum (ones column of v gives the
    # softmax denominator in row 64), evacuate rows 0..63 to the head-pair
    # outP tile via ACT and the denominator row straight to partition 32*h4 of
    # the group's den4 tile. One DVE reciprocal per 4 heads; the PE broadcasts
    # each head's recip row into its 64-partition half of a [128,570] psum and
    # one DVE mul normalizes the pair in place.
    outP = [opool.tile([128, N1P], BF16, tag="outP", name="outP") for _ in range(4)]
    for g in range(2):
        for h4 in range(4):
            h = g * 4 + h4
            for mc in range(5):
                c0, cw = CHUNKS[mc]
                et = expT[(h, mc)]
                if mc == 0:
                    nc.gpsimd.tensor_mul(
                        out=et[:cw, SEQ:N1], in0=et[:cw, SEQ:N1], in1=isone[mc])
                elif mc == 1:
                    nc.vector.tensor_mul(
                        out=et[:cw, SEQ:N1], in0=et[:cw, SEQ:N1], in1=isone[mc])
                else:
                    nc.vector.tensor_mul(
                        out=et[:cw, 0:SEQ], in0=et[:cw, 0:SEQ], in1=isoT[mc - 2][:cw, :])
            pso = ps_big.tile([65, N1P], F32, tag="big", name="psout")
            for s0, sw in SPANS:
                for mc in range(5):
                    c0, cw = CHUNKS[mc]
                    nc.tensor.matmul(
                        out=pso[:, s0:s0 + sw],
                        lhsT=v_sb[mc][:cw, h, :],
                        rhs=expT[(h, mc)][:cw, s0:s0 + sw],
                        start=(mc == 0), stop=(mc == 4),
                    )
            with nc.allow_low_precision(reason="bf16 evac"):
                nc.scalar.activation(
                    out=outP[h // 2][64 * (h % 2):64 * (h % 2) + 64, :],
                    in_=pso[0:64, :], func=AFT.Copy)
            nc.scalar.activation(
                out=den4[g][32 * h4:32 * h4 + 1, :],
                in_=pso[64:65, :], func=AFT.Copy)
        nc.vector.reciprocal_approx_fast(out=drec_f, in_=den4[g])
        with nc.allow_low_precision(reason="bf16 recip"):
            nc.vector.tensor_copy(out=drec[g], in_=drec_f)
        for jj in range(2):
            pairidx = 2 * g + jj
            psb = ps_nrm.tile([128, N1P], F32, tag="psb", name="psb")
            # the pair's two row groups (32*h4 vs 32*h4+32) are disjoint, so
            # emitting the two heads' matmuls adjacently per span runs them
            # concurrently on the array
            for s0, sw in SPANS:
                for hh in range(2):
                    h4 = 2 * jj + hh
                    nc.tensor.matmul(
                        out=psb[64 * hh:64 * hh + 64, s0:s0 + sw],
                        lhsT=ones_sb[32 * h4:32 * h4 + 1, :],
                        rhs=drec[g][32 * h4:32 * h4 + 1, s0:s0 + sw],
                        start=True, stop=True,
                        tile_position=(32 * h4, 64 * hh),
                    )
            with nc.allow_low_precision(reason="in-place normalize"):
                nc.vector.tensor_mul(out=outP[pairidx], in0=outP[pairidx], in1=psb)

    # ---- o_proj + bias + store ----
    for mc in range(5):
        c0, cw = CHUNKS[mc]
        cwp = CWP[mc]
        psf = ps_a.tile([128, E], F32, tag="psa", name="psf")
        for j in range(4):
            nc.tensor.matmul(
                out=psf[:cwp, :],
                lhsT=outP[j][:, c0:c0 + cwp],
                rhs=owP[:, j, :],
                start=(j == 0), stop=(j == 3),
            )
        fin = spool.tile([128, E], BF16, tag="fin", name="fin")
        nc.vector.tensor_add(out=fin[:cw, :], in0=psf[:cw, :], in1=ob_bc[:cw, :])
        nc.sync.dma_start(out=d_out[c0:c0 + cw, :], in_=fin[:cw, :])

    ctx.close()


def _constants():
    # block-diag: bd[w][r, s'] = 1 iff s' == (r//16)*16 + w; duplicated in
    # pairs for DoubleRow (both elements of a column pair share the map).
    # Packed host-side as [128 partitions, 16*2*128] so the load is one
    # clean 4KB-per-partition transfer.
    bd = np.zeros((128, 16, 2, 128), dtype=np.float32)
    r = np.arange(128)
    for w in range(16):
        bd[r, w, 0, (r // 16) * 16 + w] = 1.0
        bd[r, w, 1, (r // 16) * 16 + w] = 1.0
    ident = np.eye(128, dtype=ml_dtypes.bfloat16)
    return bd.reshape(128, 16 * 256).astype(ml_dtypes.float8_e4m3), ident


def kernel(x, colors, mask, qkv_w, o_w, o_b, tau):
    global LAST_RESULT
    if "nc" not in _CACHED:
        _CACHED["nc"] = _build_program()
    nc = _CACHED["nc"]

    bd, ident = _constants()
    # pack weight layouts to match SBUF tiles exactly: [part, chunk, col]
    qkv_wT = np.asarray(qkv_w, dtype=np.float32).T.astype(ml_dtypes.bfloat16)
    qkv_wT = np.ascontiguousarray(
        qkv_wT.reshape(4, 128, 3 * E).transpose(1, 0, 2)).reshape(128, 4 * 3 * E)
    # o_w as head-pair blocks: pair j rows 0:64 = head 2j, 64:128 = head 2j+1
    o_wT = np.asarray(o_w, dtype=np.float32).T.astype(ml_dtypes.bfloat16)
    o_wP = np.ascontiguousarray(
        o_wT.reshape(4, 128, E).transpose(1, 0, 2)).reshape(128, 4 * E)
    o_b2 = np.asarray(o_b, dtype=np.float32).reshape(1, E)
    tau2 = np.asarray(tau, dtype=np.float32).reshape(1, 1)

    # mask values are exactly 0.0/1.0 -> cast to fp8 is lossless and quarters
    # the HBM stream; pad the c dim to 320 so DoubleRow pair strides are
    # 16B-aligned
    m8 = np.zeros((B, IMG, IMG, NCP), dtype=ml_dtypes.float8_e4m3)
    m8[..., :NCLS] = np.asarray(mask, dtype=np.float32).astype(ml_dtypes.float8_e4m3)

    in_maps = []
    for b in range(B):
        xTf = np.concatenate([np.asarray(x[b]), np.asarray(colors[b])],
                             axis=0).T.astype(ml_dtypes.bfloat16)
        xT = np.zeros((128, 4, N1P), dtype=ml_dtypes.bfloat16)
        xT[:, :, :N1] = xTf.reshape(4, 128, N1).transpose(1, 0, 2)
        xT = xT.reshape(128, 4 * N1P)
        mb = m8[b].reshape(IMG, IMG * NCP)
        in_maps.append({
            "xT": xT, "mask": mb, "qkv_wT": qkv_wT, "o_wP": o_wP,
            "o_b": o_b2, "tau": tau2, "bd": bd, "ident": ident,
        })

    res = run_bass_kernel_spmd(nc, in_maps, list(range(B)))
    LAST_RESULT = res
    out = np.stack([res.results[i]["out"] for i in range(B)]).astype(np.float32)
    return out


# revision 23
# speedup vs baseline: 1.0380x; 1.0380x over previous
"""ColorAttention Trainium2 kernel.

Data-parallel over batch: core b handles batch element b.
Per core:
  - mask [256,256,313] is cast to fp8 (0/1 values, lossless) on the host and
    streamed from HBM (20.9MB with c padded to 320), then patch-reduced via
    block-diagonal ones matmuls on the PE in fp8 DoubleRow mode (2 image
    columns per PE cycle, PSUM accumulation), giving m[s,c] = sum over 16x16
    patch. Multiplicative attention mask is_one(m) = relu(1-(m-1)^2)
    (exact for integer m: 1 iff m==1).
  - attention computed in transposed layout throughout:
      qkvT[f,n] = sum_e qkv_wT[e,f] * inputsT[e,n]
      scoresT[m,n] = sum_d kT[d,m] qT[d,n];  expT = exp(scoresT/tau) * mask
      outT_aug[d|1,n] = sum_m v_aug[m,d|1] expT[m,n]   (row 64 = denom)
      out[n,g] = (sum_{h,d} (outT_h/denom_h)[d,n] o_wT[h*64+d,g]) + o_b
  - all attention matmuls in bf16 (1 cyc/col at any width); heads packed in
    pairs on the 128 partitions for normalize / o_proj.
  - setup DMAs ride the idle SP HWDGE ring (bd first) so the ACT engine is
    free for exp and the mask stream (gpsimd SWDGE ring) is unobstructed.
  - per-head softmax denominators are ACT-copied from psum row 64 straight to
    partitions {0,32,64,96} of a gather tile; one DVE reciprocal per 4 heads;
    PE broadcasts each recip row into the matching 64-partition half of a
    [128,570] psum so one DVE mul normalizes a head pair in place.
"""

import numpy as np
import ml_dtypes

# tolerate environments without the optional NTFF profile hook module when
# BASS_TRACE is set externally
try:
    import antenv.axon_hooks  # noqa: F401
except Exception:
    import sys as _sys
    import types as _types
    _m = _types.ModuleType("antenv.axon_hooks")
    _m.set_axon_ntff_profile_hook = lambda h: None
    _m.get_axon_ntff_profile_hook = lambda: None
    try:
        import antenv
        antenv.axon_hooks = _m
        _sys.modules["antenv.axon_hooks"] = _m
    except Exception:
        pass

import concourse.bass as bass
import concourse.mybir as mybir
import concourse.tile as tile
from concourse import bacc
from concourse.bass_utils import run_bass_kernel_spmd

F32 = mybir.dt.float32
F32R = mybir.dt.float32r
BF16 = mybir.dt.bfloat16
FP8 = mybir.dt.float8e4
AFT = mybir.ActivationFunctionType
DR = mybir.MatmulPerfMode.DoubleRow

B = 8
SEQ = 256
NCLS = 313
NCP = 320  # c dim padded to a 16B multiple so fp8 DoubleRow strides are legal
E = 512
NH = 8
HD = 64
N1 = SEQ + NCLS  # 569
P = 16
IMG = 256

# n/m chunking of the 569 token dim.
N1P = 570
CHUNKS = [(0, 128), (128, 128), (256, 128), (384, 128), (512, 57)]
CWP = [128, 128, 128, 128, 58]
SPANS = [(0, 512), (512, 58)]

# mask stream tiling: (row_block, col0, width). Small lead-in tiles so the
# first PE work starts early; 64-col (2.62MB) tiles once the pipe is primed.
# The first four tiles ride the sync HWDGE ring (which starves the SWDGE ring
# while active, so urgent data must go there); the rest stream on SWDGE.
TILES = [
    (0, 0, 16), (0, 16, 16), (0, 32, 32), (0, 64, 64), (0, 128, 64),
    (0, 192, 64),
    (1, 0, 64), (1, 64, 64), (1, 128, 64), (1, 192, 64),
]
N_SYNC_TILES = 6
# attention work units interleaved after each tile's matmuls (33 total)
BUDGET = [0, 0, 2, 3, 5, 5, 5, 5, 5, 3]
# scr-based keep-warm matmuls after early tiles (no data deps)
FILLER = [2, 3, 0, 0, 0, 0, 0, 0, 0, 0]

LAST_RESULT = None
_CACHED = {}


def r32(ap):
    if ap.dtype == F32R:
        return ap
    return ap.bitcast(F32R)


def _build_program():
    nc = bacc.Bacc("TRN2", target_bir_lowering=False, debug=False, num_devices=B)

    # ---- DRAM I/O ----
    d_xT = nc.dram_tensor("xT", [128, 4 * N1P], BF16, kind="ExternalInput").ap()
    d_mask = nc.dram_tensor("mask", [IMG, IMG * NCP], FP8, kind="ExternalInput").ap()
    d_qkvwT = nc.dram_tensor("qkv_wT", [128, 4 * 3 * E], BF16, kind="ExternalInput").ap()
    d_owP = nc.dram_tensor("o_wP", [128, 4 * E], BF16, kind="ExternalInput").ap()
    d_ob = nc.dram_tensor("o_b", [1, E], F32, kind="ExternalInput").ap()
    d_tau = nc.dram_tensor("tau", [1, 1], F32, kind="ExternalInput").ap()
    d_bd = nc.dram_tensor("bd", [128, 16 * 256], FP8, kind="ExternalInput").ap()
    d_ident = nc.dram_tensor("ident", [128, 128], BF16, kind="ExternalInput").ap()
    d_out = nc.dram_tensor("out", [N1, E], BF16, kind="ExternalOutput").ap()

    with tile.TileContext(nc) as tc:
        _emit(nc, tc, d_xT, d_mask, d_qkvwT, d_owP, d_ob, d_tau, d_bd, d_ident, d_out)

    nc.compile()
    return nc


def _emit(nc, tc, d_xT, d_mask, d_qkvwT, d_owP, d_ob, d_tau, d_bd, d_ident, d_out):
    from contextlib import ExitStack

    ctx = ExitStack()
    singles = ctx.enter_context(tc.tile_pool(name="singles", bufs=1))
    expool = ctx.enter_context(tc.tile_pool(name="expT", bufs=40))
    opool = ctx.enter_context(tc.tile_pool(name="outTsb", bufs=4))
    spool = ctx.enter_context(tc.tile_pool(name="smalls", bufs=2))
    ps_a = ctx.enter_context(tc.tile_pool(name="ps_a", bufs=2, space="PSUM"))
    ps_big = ctx.enter_context(tc.tile_pool(name="ps_big", bufs=2, space="PSUM"))
    mctx = ExitStack()
    mh_small = mctx.enter_context(tc.tile_pool(name="mh_small", bufs=2))
    mh_mid = mctx.enter_context(tc.tile_pool(name="mh_mid", bufs=1))
    mpool = mctx.enter_context(tc.tile_pool(name="mask_stream", bufs=3))
    ps_mask = mctx.enter_context(tc.tile_pool(name="ps_mask", bufs=2, space="PSUM"))

    # ---- persistent SBUF ----
    inputsT4 = singles.tile([128, 4, N1P], BF16, tag="inT", name="inputsT4")
    inputsT = [inputsT4[:, i, :] for i in range(4)]
    qkvwT4 = singles.tile([128, 4, 3 * E], BF16, tag="qkvwT", name="qkvwT4")
    qkvwT = [qkvwT4[:, i, :] for i in range(4)]
    owP = singles.tile([128, 4, E], BF16, tag="owP", name="owP")
    bd_sb = singles.tile([128, 16, 2, 128], FP8, tag="bd", name="bd_sb")
    ident_sb = singles.tile([128, 128], BF16, tag="ident", name="ident_sb")
    ones_sb = singles.tile([128, 64], BF16, tag="ones", name="ones_sb")
    rtau = singles.tile([128, 1], F32, tag="rtau", name="rtau")
    ob_bc = singles.tile([128, E], F32, tag="ob", name="ob_bc")
    qkT = [singles.tile([128, N1P], BF16, tag=f"qkT{i}", name=f"qkT{i}") for i in range(8)]
    v_sb = [singles.tile([128, NH, HD + 1], BF16, tag=f"vsb{i}", name=f"v_sb{i}") for i in range(5)]
    isone = [singles.tile([128, NCLS], BF16, tag=f"iso{i}", name=f"isone{i}") for i in range(2)]
    isoT = [singles.tile([128, SEQ], BF16, tag=f"isoT{i}", name=f"isoT{i}") for i in range(3)]
    den4 = [singles.tile([128, N1P], F32, tag=f"den{g}", name=f"den4_{g}") for g in range(2)]
    drec_f = singles.tile([128, N1P], F32, tag="drecf", name="drec_f")
    drec = [singles.tile([128, N1P], BF16, tag=f"drec{g}", name=f"drec{g}") for g in range(2)]

    # ---- short HAM warmup: keep the PE busy while the setup DMAs and the
    # first mask tiles land (the HAM SHORT window needs ~3.4us of activity) ----
    scr = singles.tile([128, 640], BF16, tag="scr", name="scr")
    nc.vector.memset(scr, 1.0)
    ps_warm = ps_a.tile([128, 512], F32, tag="psa", name="ps_warm")
    for _ in range(6):
        nc.tensor.matmul(out=ps_warm, lhsT=scr[:, 0:128], rhs=scr[:, 128:640],
                         start=True, stop=True)

    # ---- the two broadcast loads (tau, o_b need partition-replication ->
    # SWDGE) lead the gpsimd ring ----
    tau_bc = bass.AP(tensor=d_tau.tensor, offset=d_tau.offset, ap=[[0, 128], [1, 1]])
    tau_sb = singles.tile([128, 1], F32, tag="tau", name="tau_sb")
    nc.gpsimd.dma_start(out=tau_sb, in_=tau_bc)
    ob_src = bass.AP(tensor=d_ob.tensor, offset=d_ob.offset, ap=[[0, 128], [1, E]])
    nc.gpsimd.dma_start(out=ob_bc, in_=ob_src)
    nc.vector.reciprocal(out=rtau, in_=tau_sb)
    nc.vector.memset(ones_sb, 1.0)
    neg1 = singles.tile([128, 1], F32, tag="neg1", name="neg1")
    nc.vector.memset(neg1, -1.0)

    def mask_tile_dma(idx):
        rt, c0, w = TILES[idx]
        if w == 16:
            t = mh_small.tile([128, 1, 8, 2, NCP], FP8, tag="mh_s", name="mh_s")
        elif w == 32:
            t = mh_mid.tile([128, 2, 8, 2, NCP], FP8, tag="mh_m", name="mh_m")
        else:
            t = mpool.tile([128, 4, 8, 2, NCP], FP8, tag="mstream", name="mstream")
        src = bass.AP(
            tensor=d_mask.tensor,
            offset=d_mask.offset + rt * 128 * IMG * NCP + c0 * NCP,
            ap=[[IMG * NCP, 128], [1, w * NCP]],
        )
        eng = nc.sync if idx < N_SYNC_TILES else nc.gpsimd
        eng.dma_start(out=t[:, : w // 16], in_=src)
        return t

    # ---- DMA issue order. The two rings share SDMA bandwidth roughly
    # fairly, so everything needed early rides the sync HWDGE ring in exact
    # consumption order; the SWDGE ring carries only the steady-state tail
    # tiles, whose issue is naturally delayed by mask-pool slot recycling.
    pre = {}
    nc.sync.dma_start(out=bd_sb, in_=d_bd)
    pre[0] = mask_tile_dma(0)
    pre[1] = mask_tile_dma(1)
    nc.sync.dma_start(out=inputsT4, in_=d_xT)
    nc.sync.dma_start(out=qkvwT4, in_=d_qkvwT)
    pre[2] = mask_tile_dma(2)
    pre[3] = mask_tile_dma(3)
    # owP/ident are consumed late -> keep them off the sync ring's critical
    # prefix (they trickle in on the SWDGE ring while it is otherwise idle)
    nc.gpsimd.dma_start(out=owP, in_=d_owP)
    nc.gpsimd.dma_start(out=ident_sb, in_=d_ident)
    pre[4] = mask_tile_dma(4)
    pre[5] = mask_tile_dma(5)

    # ---- attention work units (emitted interleaved with the mask stream) ----
    expT = {}

    def unit_qkvT(fc):
        def go():
            for s0, sw in SPANS:
                ps = ps_a.tile([128, sw], F32, tag="psa", name="pswork")
                for ec in range(4):
                    nc.tensor.matmul(
                        out=ps,
                        lhsT=qkvwT[ec][:, fc * 128:(fc + 1) * 128],
                        rhs=inputsT[ec][:, s0:s0 + sw],
                        start=(ec == 0), stop=(ec == 3),
                    )
                with nc.allow_low_precision(reason="bf16 qk"):
                    nc.vector.tensor_copy(out=qkT[fc][:, s0:s0 + sw], in_=ps)
        return go

    def unit_v(mc):
        def go():
            c0, cw = CHUNKS[mc]
            cwp = CWP[mc]
            ps = ps_a.tile([128, NH, HD], F32, tag="psa", name="pswork")
            for ec in range(4):
                nc.tensor.matmul(
                    out=ps[:cwp],
                    lhsT=inputsT[ec][:, c0:c0 + cwp],
                    rhs=qkvwT[ec][:, 2 * E:3 * E],
                    start=(ec == 0), stop=(ec == 3),
                )
            with nc.allow_low_precision(reason="bf16 v"):
                nc.vector.tensor_copy(out=v_sb[mc][:cw, :, 0:HD], in_=ps[:cw])
            nc.vector.memset(v_sb[mc][:cw, :, HD:HD + 1], 1.0)
        return go

    def unit_scores_pair(q, mc):
        # heads 2q (PE rows 0:64) and 2q+1 (rows 64:128): the two matmuls of a
        # span are emitted adjacently so their disjoint row groups run
        # concurrently on the array.
        def go():
            c0, cw = CHUNKS[mc]
            cwp = CWP[mc]
            kt = qkT[4 + q]
            qt = qkT[q]
            ets = []
            pss = []
            for hh in range(2):
                et = expool.tile([128, N1P], BF16, tag="expT", name="expT")
                expT[(2 * q + hh, mc)] = et
                ets.append(et)
                pss.append(ps_big.tile([128, N1P], F32, tag="big", name="ps_sc"))
            for s0, sw in SPANS:
                for hh in range(2):
                    hb = 64 * hh
                    nc.tensor.matmul(
                        out=pss[hh][:cwp, s0:s0 + sw],
                        lhsT=kt[hb:hb + 64, c0:c0 + cwp],
                        rhs=qt[hb:hb + 64, s0:s0 + sw],
                        start=True, stop=True,
                    )
            for hh in range(2):
                nc.scalar.activation(
                    out=ets[hh][:cwp, :], in_=pss[hh][:cwp, :],
                    func=AFT.Exp, scale=rtau[:cwp],
                )
        return go

    # interleave so scores (ACT exp) work spreads across the whole stream
    units = []
    for q in range(4):
        units.append(unit_qkvT(q))
        units.append(unit_qkvT(4 + q))
        units.append(unit_v(q))
        for mc in range(5):
            units.append(unit_scores_pair(q, mc))
    units.append(unit_v(4))

    # ---- is_one computation (psum -> multiplicative mask) ----
    ps_m = [None, None]

    def emit_isone(i):
        tmp = spool.tile([128, NCLS], F32, tag="isotmp", name="isotmp")
        nc.scalar.activation(out=tmp, in_=ps_m[i], func=AFT.Square, bias=neg1)
        nc.scalar.activation(out=isone[i], in_=tmp, func=AFT.Relu, scale=-1.0, bias=1.0)

    def emit_isoT(i):
        # transpose is_one -> isoT (c on partitions); half i fills columns
        # i*128..i*128+128
        for j in range(3):
            cw = 57 if j == 2 else 128
            pst = ps_a.tile([128, 128], BF16, tag="psa", name="pswork_t")
            nc.tensor.transpose(out=pst[:cw, :], in_=isone[i][:, j * 128:j * 128 + cw],
                                identity=ident_sb)
            nc.vector.tensor_copy(out=isoT[j][:cw, i * 128:(i + 1) * 128], in_=pst[:cw, :])

    # ---- the mask stream: fp8 tiles of [128 rows, w cols x 320c].
    # DoubleRow pairs adjacent image columns; all matmuls of a row-block
    # accumulate the patch sum into ps_m[rt][s, c].
    ui = 0
    last_rt = -1
    for idx, (rt, c0, w) in enumerate(TILES):
        if rt != last_rt:
            ps_m[rt] = ps_mask.tile([128, NCLS], F32, tag="psmask", name="psmask")
            last_rt = rt
        t = pre.pop(idx, None)
        if t is None:
            t = mask_tile_dma(idx)
        first_tile = (c0 == 0)
        last_tile = (c0 + w == IMG)
        for wi in range(w // 16):
            wg = c0 // 16 + wi
            for jp in range(8):
                nc.tensor.matmul(
                    out=ps_m[rt],
                    lhsT=bd_sb[:, wg, :, :],
                    rhs=t[:, wi, jp, :, :NCLS],
                    start=(first_tile and wi == 0 and jp == 0),
                    stop=(last_tile and wi == w // 16 - 1 and jp == 7),
                    perf_mode=DR,
                )
        for _ in range(FILLER[idx]):
            pf = ps_a.tile([128, 512], F32, tag="psa", name="pf")
            nc.tensor.matmul(out=pf, lhsT=scr[:, 0:128], rhs=scr[:, 128:640],
                             start=True, stop=True)
        if last_tile:
            # is_one first so its ACT ops aren't queued behind the units' exps
            emit_isone(rt)
        for _ in range(BUDGET[idx]):
            if ui < len(units):
                units[ui]()
                ui += 1
        if last_tile:
            emit_isoT(rt)
    while ui < len(units):
        units[ui]()
        ui += 1
    mctx.close()
    ps_nrm = ctx.enter_context(tc.tile_pool(name="ps_nrm", bufs=1, space="PSUM"))

    # ---- mask-mult + attn@v + normalize (head pairs) ----
    # Per head: multiply expT by the mask (color-key side on gpsimd, patch-key
    # side on DVE), attn@v into a 2-bank psum (ones column of v gives the
    # softmax denominator in row 64), evacuate rows 0..63 to the head-pair
    # outP tile via ACT and the denominator row straight to partition 32*h4 of
    # the group's den4 tile. One DVE reciprocal per 4 heads; the PE broadcasts
    # each head's recip row into its 64-partition half of a [128,570] psum and
    # one DVE mul normalizes the pair in place.
    outP = [opool.tile([128, N1P], BF16, tag="outP", name="outP") for _ in range(4)]
    for g in range(2):
        for h4 in range(4):
            h = g * 4 + h4
            for mc in range(5):
                c0, cw = CHUNKS[mc]
                et = expT[(h, mc)]
                if mc == 0:
                    nc.gpsimd.tensor_mul(
                        out=et[:cw, SEQ:N1], in0=et[:cw, SEQ:N1], in1=isone[mc])
                elif mc == 1:
                    nc.vector.tensor_mul(
                        out=et[:cw, SEQ:N1], in0=et[:cw, SEQ:N1], in1=isone[mc])
                else:
                    nc.vector.tensor_mul(
                        out=et[:cw, 0:SEQ], in0=et[:cw, 0:SEQ], in1=isoT[mc - 2][:cw, :])
            pso = ps_big.tile([65, N1P], F32, tag="big", name="psout")
            for s0, sw in SPANS:
                for mc in range(5):
                    c0, cw = CHUNKS[mc]
                    nc.tensor.matmul(
                        out=pso[:, s0:s0 + sw],
                        lhsT=v_sb[mc][:cw, h, :],
                        rhs=expT[(h, mc)][:cw, s0:s0 + sw],
                        start=(mc == 0), stop=(mc == 4),
                    )
            with nc.allow_low_precision(reason="bf16 evac"):
                nc.scalar.activation(
                    out=outP[h // 2][64 * (h % 2):64 * (h % 2) + 64, :],
                    in_=pso[0:64, :], func=AFT.Copy)
            nc.scalar.activation(
                out=den4[g][32 * h4:32 * h4 + 1, :],
                in_=pso[64:65, :], func=AFT.Copy)
        nc.vector.reciprocal_approx_fast(out=drec_f, in_=den4[g])
        with nc.allow_low_precision(reason="bf16 recip"):
            nc.vector.tensor_copy(out=drec[g], in_=drec_f)
        for jj in range(2):
            pairidx = 2 * g + jj
            psb = ps_nrm.tile([128, N1P], F32, tag="psb", name="psb")
            # the pair's two row groups (32*h4 vs 32*h4+32) are disjoint, so
            # emitting the two heads' matmuls adjacently per span runs them
            # concurrently on the array
            for s0, sw in SPANS:
                for hh in range(2):
                    h4 = 2 * jj + hh
                    nc.tensor.matmul(
                        out=psb[64 * hh:64 * hh + 64, s0:s0 + sw],
                        lhsT=ones_sb[32 * h4:32 * h4 + 1, :],
                        rhs=drec[g][32 * h4:32 * h4 + 1, s0:s0 + sw],
                        start=True, stop=True,
                        tile_position=(32 * h4, 64 * hh),
                    )
            with nc.allow_low_precision(reason="in-place normalize"):
                nc.vector.tensor_mul(out=outP[pairidx], in0=outP[pairidx], in1=psb)

    # ---- o_proj + bias + store ----
    for mc in range(5):
        c0, cw = CHUNKS[mc]
        cwp = CWP[mc]
        psf = ps_a.tile([128, E], F32, tag="psa", name="psf")
        for j in range(4):
            nc.tensor.matmul(
                out=psf[:cwp, :],
                lhsT=outP[j][:, c0:c0 + cwp],
                rhs=owP[:, j, :],
                start=(j == 0), stop=(j == 3),
            )
        fin = spool.tile([128, E], BF16, tag="fin", name="fin")
        nc.vector.tensor_add(out=fin[:cw, :], in0=psf[:cw, :], in1=ob_bc[:cw, :])
        nc.sync.dma_start(out=d_out[c0:c0 + cw, :], in_=fin[:cw, :])

    ctx.close()


def _constants():
    # block-diag: bd[w][r, s'] = 1 iff s' == (r//16)*16 + w; duplicated in
    # pairs for DoubleRow (both elements of a column pair share the map).
    # Packed host-side as [128 partitions, 16*2*128] so the load is one
    # clean 4KB-per-partition transfer.
    bd = np.zeros((128, 16, 2, 128), dtype=np.float32)
    r = np.arange(128)
    for w in range(16):
        bd[r, w, 0, (r // 16) * 16 + w] = 1.0
        bd[r, w, 1, (r // 16) * 16 + w] = 1.0
    ident = np.eye(128, dtype=ml_dtypes.bfloat16)
    return bd.reshape(128, 16 * 256).astype(ml_dtypes.float8_e4m3), ident


def kernel(x, colors, mask, qkv_w, o_w, o_b, tau):
    global LAST_RESULT
    if "nc" not in _CACHED:
        _CACHED["nc"] = _build_program()
    nc = _CACHED["nc"]

    bd, ident = _constants()
    # pack weight layouts to match SBUF tiles exactly: [part, chunk, col]
    qkv_wT = np.asarray(qkv_w, dtype=np.float32).T.astype(ml_dtypes.bfloat16)
    qkv_wT = np.ascontiguousarray(
        qkv_wT.reshape(4, 128, 3 * E).transpose(1, 0, 2)).reshape(128, 4 * 3 * E)
    # o_w as head-pair blocks: pair j rows 0:64 = head 2j, 64:128 = head 2j+1
    o_wT = np.asarray(o_w, dtype=np.float32).T.astype(ml_dtypes.bfloat16)
    o_wP = np.ascontiguousarray(
        o_wT.reshape(4, 128, E).transpose(1, 0, 2)).reshape(128, 4 * E)
    o_b2 = np.asarray(o_b, dtype=np.float32).reshape(1, E)
    tau2 = np.asarray(tau, dtype=np.float32).reshape(1, 1)

    # mask values are exactly 0.0/1.0 -> cast to fp8 is lossless and quarters
    # the HBM stream; pad the c dim to 320 so DoubleRow pair strides are
    # 16B-aligned
    m8 = np.zeros((B, IMG, IMG, NCP), dtype=ml_dtypes.float8_e4m3)
    m8[..., :NCLS] = np.asarray(mask, dtype=np.float32).astype(ml_dtypes.float8_e4m3)

    in_maps = []
    for b in range(B):
        xTf = np.concatenate([np.asarray(x[b]), np.asarray(colors[b])],
                             axis=0).T.astype(ml_dtypes.bfloat16)
        xT = np.zeros((128, 4, N1P), dtype=ml_dtypes.bfloat16)
        xT[:, :, :N1] = xTf.reshape(4, 128, N1).transpose(1, 0, 2)
        xT = xT.reshape(128, 4 * N1P)
        mb = m8[b].reshape(IMG, IMG * NCP)
        in_maps.append({
            "xT": xT, "mask": mb, "qkv_wT": qkv_wT, "o_wP": o_wP,
            "o_b": o_b2, "tau": tau2, "bd": bd, "ident": ident,
        })

    res = run_bass_kernel_spmd(nc, in_maps, list(range(B)))
    LAST_RESULT = res
    out = np.stack([res.results[i]["out"] for i in range(B)]).astype(np.float32)
    return out
